# revision 18
# baseline (speedup 1.0000x reference)
"""Trainium2 Bass kernel for nn_ExactModel_15092515078731.

Reference computes, per timestep t:
    U = expm(-i t H);  psi = U[:, 0]
    rotate psi by 32 per-observable tensor-product single-qubit bases
    probs = |rotated|^2 ; gather at indices

Algorithm here: Krylov (Lanczos) projection.  H is real-symmetric, so
psi_t = expm(-itH) e0 ~= V exp(-itT) e1 with V the (t-independent!)
m=32-vector Lanczos basis of K(H, e0) and T the 32x32 tridiagonal
projection, both built on host (the baseline already ran 80 host
Lanczos iterations just for spectral bounds; this reuses that work).
Per-t coefficient vectors y_t = exp(-itT) e1 are tiny (32 complex).

Device work per core (SPMD over 8 cores, sharded by OBSERVABLE --
each core owns 4 of the 32 observables for all 8 timesteps):
  1. evolution: psi_{t,r}[p, q] = sum_k V[(q<<7)|p, k] y^r_t[k] for all
     8 t and r in {re, im, -im, re} -- 8 fp16 matmuls of 128 cols via a
     block-diagonal y trick (4 q-values x 32 k on the contraction
     partitions).
  2. rotation stage A (SWAPPED operands: state stationary, weights
     moving -- no transposes needed): psA_t[(r q), (b p')] accumulates
     cat1_t^T Wre + cat2_t^T Wim = [T_re; T_im] stacked on partitions,
     2 matmuls of 512 cols per t.
  3. rotation stage B: 64x64 complex-structured Wfree block per b,
     rhs = fp16 cast of psA in [64, (b t p')] layout; out [64, (t p')]
     = [F_re; F_im].
  4. |.|^2: square on ACT/DVE/Pool; partition pair-add moved to the PE
     as accumulating 0/1 "pair-sum" matmuls that also pack the 4
     observables onto 128 partitions for a full-width output DMA.
Host does only small parameter prep (Lanczos on one 4096-vector,
rotation kron products) and the final index gather.
"""
import sys

if "/opt/trn_rl_repo" not in sys.path:
    sys.path.insert(0, "/opt/trn_rl_repo")

from contextlib import ExitStack

import numpy as np

import concourse.bacc as bacc
import concourse.bass as bass  # noqa: F401
import concourse.mybir as mybir
import concourse.tile as tile
from concourse.bass_utils import run_bass_kernel_spmd

N = 12
DIM = 4096
P = 128    # partition: bits 0-6
F = 32     # free: bits 7-11
NCORES = 8
B = 32     # observables
BPC = 4    # observables per core
T = 8      # timesteps
M = 32     # Krylov dimension

_s = 1.0 / np.sqrt(2.0)
U_BASIS = np.stack([
    np.array([[1, 1], [1, -1]]) * _s,
    np.array([[1, -1j], [1, 1j]]) * _s,
    np.eye(2),
]).astype(np.complex128)

F32 = mybir.dt.float32
F16 = mybir.dt.float16
MULT = mybir.AluOpType.mult
ADD = mybir.AluOpType.add


# ----------------------------------------------------------------------------
# host math
# ----------------------------------------------------------------------------

def _build_zz_diag(params_zz):
    basis = np.arange(DIM)
    bits = (basis[:, None] >> np.arange(N)[None, :]) & 1
    signs = (1 - 2 * bits).astype(np.float64)
    return (signs[:, :-1] * signs[:, 1:]) @ params_zz


def _h_matvec(v, params_x, zz_diag):
    out = zz_diag * v
    idx = np.arange(DIM)
    for i in range(N):
        out = out + params_x[i] * v[idx ^ (1 << i)]
    return out


def _lanczos(params_x, zz_diag, m=M):
    """m-step Lanczos of H from e0 with full reorthogonalization.
    Returns V (DIM, m).  On breakdown the remaining columns stay zero
    (the Krylov space is then invariant and the projection exact)."""
    V = np.zeros((DIM, m))
    V[0, 0] = 1.0
    for j in range(m - 1):
        w = _h_matvec(V[:, j], params_x, zz_diag)
        for _ in range(2):
            w = w - V[:, :j + 1] @ (V[:, :j + 1].T @ w)
        beta = np.linalg.norm(w)
        if beta < 1e-10:
            break
        V[:, j + 1] = w / beta
    return V


def _build_rot_mats(pauli_obs):
    """Wpart (B,128,128), Wfree (B,32,32); qubit acting on bit k is
    U_BASIS[pauli_obs[b, 11-k]] (reference reshape is bit-11-major)."""
    Wpart = np.zeros((B, P, P), np.complex128)
    Wfree = np.zeros((B, F, F), np.complex128)
    for b in range(B):
        Ub = [U_BASIS[pauli_obs[b, 11 - k]] for k in range(N)]
        wp = np.array([[1.0]])
        for k in range(6, -1, -1):
            wp = np.kron(wp, Ub[k])
        wf = np.array([[1.0]])
        for k in range(11, 6, -1):
            wf = np.kron(wf, Ub[k])
        Wpart[b] = wp
        Wfree[b] = wf
    return Wpart, Wfree


def prepare_host_data(initial_state, ts, pauli_obs, params_x, params_zz):
    """Returns (shared dict, per-core list of dicts)."""
    n0 = int(initial_state)
    assert n0 == 0
    ts = np.asarray(ts, np.float64)
    pauli_obs = np.asarray(pauli_obs, np.int64)
    params_x = np.asarray(params_x, np.float64)
    params_zz = np.asarray(params_zz, np.float64)

    zz_diag = _build_zz_diag(params_zz)
    V = _lanczos(params_x, zz_diag)                       # (DIM, M)
    HV = np.stack([_h_matvec(V[:, k], params_x, zz_diag)
                   for k in range(M)], axis=1)
    Tm = V.T @ HV                                          # (M, M)
    wT, QT = np.linalg.eigh(Tm)
    # y_t = exp(-i t T) e1
    ys = [QT @ (np.exp(-1j * t * wT) * QT[0, :]) for t in ts]

    # V in evolution lhsT layout: V16[(j,k), g*128+p] = V[((4g+j)<<7)|p, k]
    Vr = V.reshape(F, P, M)                                # [q, p, k]
    V16 = np.zeros((P, 8 * P), np.float16)
    for g in range(8):
        for j in range(4):
            # rows j*32+k, cols g*128+p
            V16[j * M:(j + 1) * M, g * P:(g + 1) * P] = \
                Vr[4 * g + j].T.astype(np.float16)
    # Y block-diag: Y[(j,k), j'*32 + t*4 + rr] = (j==j') * y^rr_t[k]
    Y16 = np.zeros((P, P), np.float16)
    for t in range(T):
        yre = ys[t].real
        yim = ys[t].imag
        for j in range(4):
            rows = np.s_[j * M:(j + 1) * M]
            Y16[rows, j * M + t * 4 + 0] = yre.astype(np.float16)
            Y16[rows, j * M + t * 4 + 1] = yim.astype(np.float16)
            Y16[rows, j * M + t * 4 + 2] = (-yim).astype(np.float16)
            Y16[rows, j * M + t * 4 + 3] = yre.astype(np.float16)
    VY = np.concatenate([Y16, V16], axis=1)                # (128, 1152)

    Wpart, Wfree = _build_rot_mats(pauli_obs)
    per_core = []
    for c in range(NCORES):
        bs = [BPC * c + i for i in range(BPC)]
        # stage A moving weights: WA[p, w*512 + bl*128 + p'] = re/im W[p',p]
        WA = np.zeros((P, 2 * BPC * P), np.float16)
        for bl, b in enumerate(bs):
            WA[:, 0 * BPC * P + bl * P:(0 * BPC + bl + 1) * P] = \
                Wpart[b].T.real.astype(np.float16)
            WA[:, 1 * BPC * P + bl * P:1 * BPC * P + (bl + 1) * P] = \
                Wpart[b].T.imag.astype(np.float16)
        # stage B stationary: complex-structured 64x64 block per bl
        WF = np.zeros((2 * F, BPC * 2 * F), np.float16)
        for bl, b in enumerate(bs):
            fre = Wfree[b].real.T.astype(np.float16)   # [q, q'] = Wf[q',q]
            fim = Wfree[b].imag.T.astype(np.float16)
            blk = np.s_[bl * 2 * F: bl * 2 * F + 2 * F]
            WF[0:F, blk][:, 0:F] = fre
            WF[0:F, blk][:, F:2 * F] = fim
            WF[F:2 * F, blk][:, 0:F] = -fim
            WF[F:2 * F, blk][:, F:2 * F] = fre
        per_core.append(dict(wa=WA, wf=WF))
    shared = dict(vy=VY)
    return shared, per_core


# ----------------------------------------------------------------------------
# device program
# ----------------------------------------------------------------------------

def _copy(eng, out, in_):
    if hasattr(eng, "tensor_copy"):
        eng.tensor_copy(out, in_)
    else:
        eng.copy(out, in_)


def build_program():
    nc = bacc.Bacc("TRN2", target_bir_lowering=False, debug=False,
                   num_devices=NCORES)

    d_vy = nc.dram_tensor("vy", [P, 9 * P], F16, kind="ExternalInput")
    d_wa = nc.dram_tensor("wa", [P, 2 * BPC * P], F16, kind="ExternalInput")
    d_wf = nc.dram_tensor("wf", [2 * F, BPC * 2 * F], F16,
                          kind="ExternalInput")
    d_sq = nc.dram_tensor("sqout", [P, 4 * BPC * P], F16,
                          kind="ExternalOutput")

    with tile.TileContext(nc) as tc, ExitStack() as ctx:
        consts = ctx.enter_context(tc.tile_pool(name="consts", bufs=1))
        work = ctx.enter_context(tc.tile_pool(name="work", bufs=1))
        sq_pool = ctx.enter_context(tc.tile_pool(name="sq", bufs=3))
        ps_psi = ctx.enter_context(tc.tile_pool(name="ps_psi", bufs=1,
                                                space="PSUM"))
        ps_a = ctx.enter_context(tc.tile_pool(name="ps_a", bufs=3,
                                              space="PSUM"))
        ps_b = ctx.enter_context(tc.tile_pool(name="ps_b", bufs=2,
                                              space="PSUM"))
        ps_j = ctx.enter_context(tc.tile_pool(name="ps_j", bufs=1,
                                              space="PSUM"))

        # dummy ACT op to trigger the activation-table load during the
        # input-DMA wait instead of at the first real ACT use
        sb_dummy = consts.tile([P, 8], F32, tag="dummy")
        nc.gpsimd.memset(sb_dummy, 0.0)
        nc.scalar.square(sb_dummy, sb_dummy)

        # input DMAs: [Y | V cols 0-383] on sync, rest of V on scalar, then
        # the rotation weights
        sb_vy = consts.tile([P, 9 * P], F16, tag="vy")
        nc.sync.dma_start(out=sb_vy[:, 0:4 * P], in_=d_vy.ap()[:, 0:4 * P])
        nc.scalar.dma_start(out=sb_vy[:, 4 * P:9 * P],
                            in_=d_vy.ap()[:, 4 * P:9 * P])
        sb_wa = consts.tile([P, 2 * BPC * P], F16, tag="wa")
        nc.scalar.dma_start(out=sb_wa, in_=d_wa.ap())
        sb_wf = consts.tile([2 * F, BPC * 2 * F], F16, tag="wf")
        nc.sync.dma_start(out=sb_wf, in_=d_wf.ap())

        # PE warm-up: junk matmuls with no data deps keep the tensor engine
        # busy through the input-DMA wait so its p-state ramps to full clock
        # before the real work arrives
        sb_junk = consts.tile([P, 4 * P], F16, tag="junk")
        nc.gpsimd.memset(sb_junk, 0.0)
        ps_junk = ps_j.tile([P, 4 * P], F32, tag="psj")
        for _ in range(6):
            nc.tensor.matmul(ps_junk[:, 0:P], sb_junk[:, 0:P],
                             sb_junk[:, 0:P],
                             start=True, stop=True, skip_group_check=True)

        # ---------------- evolution: psi for all 8 t ----------------
        # one PSUM tile, cols (g, j, t, rr); lhsT = V chunk, rhs = Y
        psi = ps_psi.tile([P, 8 * P], F32, tag="psi")
        for g in range(8):
            nc.tensor.matmul(psi[:, g * P:(g + 1) * P],
                             sb_vy[:, (1 + g) * P:(2 + g) * P],
                             sb_vy[:, 0:P], start=True, stop=True,
                             skip_group_check=True)

        # bridge junk matmuls: keep the PE busy through the rr-copy gap so
        # the DVFS ramp isn't reset before stage A
        for _ in range(4):
            nc.tensor.matmul(ps_junk, sb_junk[:, 0:P], sb_junk,
                             start=True, stop=True, skip_group_check=True)

        # rr-copies: PSUM -> fp16 cat tiles, cols (t, rr-pair, q=(g j)).
        # cat1/cat2 are separate tiles each written by ONE engine -- the tile
        # framework serializes same-tile writers across engines otherwise.
        sb_cat1 = work.tile([P, T * 2 * F], F16, tag="cat1")
        sb_cat2 = work.tile([P, T * 2 * F], F16, tag="cat2")
        psi_r = psi.rearrange("p (gj t rr) -> p rr t gj", gj=F, t=T, rr=4)
        c1_r = sb_cat1.rearrange("p (t rr gj) -> p rr t gj", gj=F, t=T, rr=2)
        c2_r = sb_cat2.rearrange("p (t rr gj) -> p rr t gj", gj=F, t=T, rr=2)
        _copy(nc.vector, c1_r[:, 0], psi_r[:, 0])
        _copy(nc.vector, c1_r[:, 1], psi_r[:, 1])
        _copy(nc.scalar, c2_r[:, 0], psi_r[:, 2])
        _copy(nc.scalar, c2_r[:, 1], psi_r[:, 3])

        # ---------------- rotation stage A ----------------
        # psA_t[(r q), (bl p')] = [T_re; T_im]; casts split per observable
        # pair into per-(h, bb) tiles, engine keyed by bb (single writer per
        # tile, both engines run concurrently per t)
        sb_a00 = work.tile([2 * F, 2 * 4 * P], F16, tag="sba00")
        sb_a01 = work.tile([2 * F, 2 * 4 * P], F16, tag="sba01")
        sb_a10 = work.tile([2 * F, 2 * 4 * P], F16, tag="sba10")
        sb_a11 = work.tile([2 * F, 2 * 4 * P], F16, tag="sba11")
        sb_ahb = [[sb_a00, sb_a01], [sb_a10, sb_a11]]
        aeng = [nc.vector, nc.scalar]

        def stage_a(t):
            psA = ps_a.tile([2 * F, BPC * P], F32, tag="psA")
            nc.tensor.matmul(psA, sb_cat1[:, t * 2 * F:(t + 1) * 2 * F],
                             sb_wa[:, 0:BPC * P], start=True, stop=False)
            nc.tensor.matmul(psA, sb_cat2[:, t * 2 * F:(t + 1) * 2 * F],
                             sb_wa[:, BPC * P:2 * BPC * P],
                             start=False, stop=True)
            psA_v = psA.rearrange("p (b x) -> p b x", b=BPC)
            for bb in range(2):
                dst = sb_ahb[t // 4][bb].rearrange(
                    "p (u t x) -> p t u x", u=2, t=4, x=P)[:, t % 4]
                _copy(aeng[bb], dst, psA_v[:, 2 * bb:2 * bb + 2])

        # ---------------- stage B + |.|^2 + out ----------------
        # psB packs two observables on the partition axis (PE quadrant
        # placement); squares on ACT; re^2+im^2 pair-add happens on host
        def stage_b(h, bb):
            psB = ps_b.tile([P, BPC * P], F32, tag="psB")
            for u in range(2):
                nc.tensor.matmul(
                    psB[64 * u:64 * (u + 1), :],
                    sb_wf[:, (2 * bb + u) * 2 * F:(2 * bb + u + 1) * 2 * F],
                    sb_ahb[h][bb][:, u * 4 * P:(u + 1) * 4 * P],
                    start=True, stop=True, skip_group_check=True)
            sq = sq_pool.tile([P, BPC * P], F16, tag="sq")
            nc.scalar.square(sq, psB)
            nc.sync.dma_start(
                out=d_sq.ap()[:, (2 * h + bb) * BPC * P:
                              (2 * h + bb + 1) * BPC * P], in_=sq)

        for t in range(4):
            stage_a(t)
        stage_a(4)
        stage_b(0, 0)
        stage_a(5)
        stage_a(6)
        stage_b(0, 1)
        stage_a(7)
        stage_b(1, 0)
        stage_b(1, 1)

    nc.compile()
    return nc


# ----------------------------------------------------------------------------
# entry point
# ----------------------------------------------------------------------------

_PROGRAM_CACHE = {}

# test-harness knobs (grading path leaves these untouched)
TRACE = False
LAST_RESULT = None


def kernel(initial_state, ts, pauli_obs, indices, params_x, params_zz):
    ts = np.asarray(ts)
    pauli_obs = np.asarray(pauli_obs)
    indices = np.asarray(indices)
    Tn = ts.shape[0]
    shots = indices.shape[2]
    assert Tn == T, f"expected {T} timesteps, got {Tn}"

    shared, per_core = prepare_host_data(
        initial_state, ts, pauli_obs, params_x, params_zz)

    if "prog" not in _PROGRAM_CACHE:
        _PROGRAM_CACHE["prog"] = build_program()
    nc = _PROGRAM_CACHE["prog"]

    in_maps = [{**shared, **pc} for pc in per_core]
    res = run_bass_kernel_spmd(nc, in_maps, core_ids=list(range(NCORES)),
                               trace=TRACE)
    global LAST_RESULT
    LAST_RESULT = res

    out = np.zeros((Tn, B, shots), np.float32)
    idx = indices.astype(np.int64)
    for c in range(NCORES):
        tiles = np.asarray(res.results[c]["sqout"], np.float32)  # (128, 2048)
        # chunk (h, bb) at cols (2h+bb)*512; rows 64u+32r+q'; cols (t%4, p')
        ch = tiles.reshape(P, 2, 2, 4, P).transpose(1, 2, 0, 3, 4)
        ch = ch.reshape(2, 2, 2, 2, F, 4, P)        # [h, bb, u, r, q', t4, p']
        pr = ch.sum(axis=3)                          # re^2 + im^2
        # -> [t, bl, n]: t = 4h + t4, bl = 2bb + u, n = q'<<7 | p'
        pr = pr.transpose(0, 4, 1, 2, 3, 5).reshape(Tn, BPC, DIM)
        for bl in range(BPC):
            b = BPC * c + bl
            out[:, b, :] = np.take_along_axis(pr[:, bl], idx[:, b], axis=1)
    return out


# revision 19
# speedup vs baseline: 1.0295x; 1.0295x over previous
"""Trainium2 Bass kernel for nn_ExactModel_15092515078731.

Reference computes, per timestep t:
    U = expm(-i t H);  psi = U[:, 0]
    rotate psi by 32 per-observable tensor-product single-qubit bases
    probs = |rotated|^2 ; gather at indices

Algorithm here: Krylov (Lanczos) projection.  H is real-symmetric, so
psi_t = expm(-itH) e0 ~= V exp(-itT) e1 with V the (t-independent!)
m=32-vector Lanczos basis of K(H, e0) and T the 32x32 tridiagonal
projection, both built on host (the baseline already ran 80 host
Lanczos iterations just for spectral bounds; this reuses that work).
Per-t coefficient vectors y_t = exp(-itT) e1 are tiny (32 complex).

Device work per core (SPMD over 8 cores, sharded by OBSERVABLE --
each core owns 4 of the 32 observables for all 8 timesteps):
  1. evolution: psi_{t,r}[p, q] = sum_k V[(q<<7)|p, k] y^r_t[k] for all
     8 t and r in {re, im, -im, re} -- 8 fp16 matmuls of 128 cols via a
     block-diagonal y trick (4 q-values x 32 k on the contraction
     partitions).
  2. rotation stage A (SWAPPED operands: state stationary, weights
     moving -- no transposes needed): psA_t[(r q), (b p')] accumulates
     cat1_t^T Wre + cat2_t^T Wim = [T_re; T_im] stacked on partitions,
     2 matmuls of 512 cols per t.
  3. rotation stage B: 64x64 complex-structured Wfree block per b,
     rhs = fp16 cast of psA in [64, (b t p')] layout; out [64, (t p')]
     = [F_re; F_im].
  4. |.|^2: square on ACT/DVE/Pool; partition pair-add moved to the PE
     as accumulating 0/1 "pair-sum" matmuls that also pack the 4
     observables onto 128 partitions for a full-width output DMA.
Host does only small parameter prep (Lanczos on one 4096-vector,
rotation kron products) and the final index gather.
"""
import sys

if "/opt/trn_rl_repo" not in sys.path:
    sys.path.insert(0, "/opt/trn_rl_repo")

from contextlib import ExitStack

import numpy as np

import concourse.bacc as bacc
import concourse.bass as bass  # noqa: F401
import concourse.mybir as mybir
import concourse.tile as tile
from concourse.bass_utils import run_bass_kernel_spmd

N = 12
DIM = 4096
P = 128    # partition: bits 0-6
F = 32     # free: bits 7-11
NCORES = 8
B = 32     # observables
BPC = 4    # observables per core
T = 8      # timesteps
M = 32     # Krylov dimension

_s = 1.0 / np.sqrt(2.0)
U_BASIS = np.stack([
    np.array([[1, 1], [1, -1]]) * _s,
    np.array([[1, -1j], [1, 1j]]) * _s,
    np.eye(2),
]).astype(np.complex128)

F32 = mybir.dt.float32
F16 = mybir.dt.float16
MULT = mybir.AluOpType.mult
ADD = mybir.AluOpType.add


# ----------------------------------------------------------------------------
# host math
# ----------------------------------------------------------------------------

def _build_zz_diag(params_zz):
    basis = np.arange(DIM)
    bits = (basis[:, None] >> np.arange(N)[None, :]) & 1
    signs = (1 - 2 * bits).astype(np.float64)
    return (signs[:, :-1] * signs[:, 1:]) @ params_zz


def _h_matvec(v, params_x, zz_diag):
    out = zz_diag * v
    idx = np.arange(DIM)
    for i in range(N):
        out = out + params_x[i] * v[idx ^ (1 << i)]
    return out


def _lanczos(params_x, zz_diag, m=M):
    """m-step Lanczos of H from e0 with full reorthogonalization.
    Returns V (DIM, m).  On breakdown the remaining columns stay zero
    (the Krylov space is then invariant and the projection exact)."""
    V = np.zeros((DIM, m))
    V[0, 0] = 1.0
    for j in range(m - 1):
        w = _h_matvec(V[:, j], params_x, zz_diag)
        for _ in range(2):
            w = w - V[:, :j + 1] @ (V[:, :j + 1].T @ w)
        beta = np.linalg.norm(w)
        if beta < 1e-10:
            break
        V[:, j + 1] = w / beta
    return V


def _build_rot_mats(pauli_obs):
    """Wpart (B,128,128), Wfree (B,32,32); qubit acting on bit k is
    U_BASIS[pauli_obs[b, 11-k]] (reference reshape is bit-11-major)."""
    Wpart = np.zeros((B, P, P), np.complex128)
    Wfree = np.zeros((B, F, F), np.complex128)
    for b in range(B):
        Ub = [U_BASIS[pauli_obs[b, 11 - k]] for k in range(N)]
        wp = np.array([[1.0]])
        for k in range(6, -1, -1):
            wp = np.kron(wp, Ub[k])
        wf = np.array([[1.0]])
        for k in range(11, 6, -1):
            wf = np.kron(wf, Ub[k])
        Wpart[b] = wp
        Wfree[b] = wf
    return Wpart, Wfree


def prepare_host_data(initial_state, ts, pauli_obs, params_x, params_zz):
    """Returns (shared dict, per-core list of dicts)."""
    n0 = int(initial_state)
    assert n0 == 0
    ts = np.asarray(ts, np.float64)
    pauli_obs = np.asarray(pauli_obs, np.int64)
    params_x = np.asarray(params_x, np.float64)
    params_zz = np.asarray(params_zz, np.float64)

    zz_diag = _build_zz_diag(params_zz)
    V = _lanczos(params_x, zz_diag)                       # (DIM, M)
    HV = np.stack([_h_matvec(V[:, k], params_x, zz_diag)
                   for k in range(M)], axis=1)
    Tm = V.T @ HV                                          # (M, M)
    wT, QT = np.linalg.eigh(Tm)
    # y_t = exp(-i t T) e1
    ys = [QT @ (np.exp(-1j * t * wT) * QT[0, :]) for t in ts]

    # V in evolution lhsT layout: V16[(j,k), g*128+p] = V[((4g+j)<<7)|p, k]
    Vr = V.reshape(F, P, M)                                # [q, p, k]
    V16 = np.zeros((P, 8 * P), np.float16)
    for g in range(8):
        for j in range(4):
            # rows j*32+k, cols g*128+p
            V16[j * M:(j + 1) * M, g * P:(g + 1) * P] = \
                Vr[4 * g + j].T.astype(np.float16)
    # Y block-diag: Y[(j,k), j'*32 + t*4 + rr] = (j==j') * y^rr_t[k]
    Y16 = np.zeros((P, P), np.float16)
    for t in range(T):
        yre = ys[t].real
        yim = ys[t].imag
        for j in range(4):
            rows = np.s_[j * M:(j + 1) * M]
            Y16[rows, j * M + t * 4 + 0] = yre.astype(np.float16)
            Y16[rows, j * M + t * 4 + 1] = yim.astype(np.float16)
            Y16[rows, j * M + t * 4 + 2] = (-yim).astype(np.float16)
            Y16[rows, j * M + t * 4 + 3] = yre.astype(np.float16)
    VY = np.concatenate([Y16, V16], axis=1)                # (128, 1152)

    Wpart, Wfree = _build_rot_mats(pauli_obs)
    per_core = []
    for c in range(NCORES):
        bs = [BPC * c + i for i in range(BPC)]
        # stage A moving weights: WA[p, w*512 + bl*128 + p'] = re/im W[p',p]
        WA = np.zeros((P, 2 * BPC * P), np.float16)
        for bl, b in enumerate(bs):
            WA[:, 0 * BPC * P + bl * P:(0 * BPC + bl + 1) * P] = \
                Wpart[b].T.real.astype(np.float16)
            WA[:, 1 * BPC * P + bl * P:1 * BPC * P + (bl + 1) * P] = \
                Wpart[b].T.imag.astype(np.float16)
        # stage B stationary: complex-structured 64x64 block per bl
        WF = np.zeros((2 * F, BPC * 2 * F), np.float16)
        for bl, b in enumerate(bs):
            fre = Wfree[b].real.T.astype(np.float16)   # [q, q'] = Wf[q',q]
            fim = Wfree[b].imag.T.astype(np.float16)
            blk = np.s_[bl * 2 * F: bl * 2 * F + 2 * F]
            WF[0:F, blk][:, 0:F] = fre
            WF[0:F, blk][:, F:2 * F] = fim
            WF[F:2 * F, blk][:, 0:F] = -fim
            WF[F:2 * F, blk][:, F:2 * F] = fre
        per_core.append(dict(wa=WA, wf=WF))
    shared = dict(vy=VY)
    return shared, per_core


# ----------------------------------------------------------------------------
# device program
# ----------------------------------------------------------------------------

def _copy(eng, out, in_):
    if hasattr(eng, "tensor_copy"):
        eng.tensor_copy(out, in_)
    else:
        eng.copy(out, in_)


def build_program():
    nc = bacc.Bacc("TRN2", target_bir_lowering=False, debug=False,
                   num_devices=NCORES)

    d_vy = nc.dram_tensor("vy", [P, 9 * P], F16, kind="ExternalInput")
    d_wa = nc.dram_tensor("wa", [P, 2 * BPC * P], F16, kind="ExternalInput")
    d_wf = nc.dram_tensor("wf", [2 * F, BPC * 2 * F], F16,
                          kind="ExternalInput")
    d_sq = nc.dram_tensor("sqout", [P, 4 * BPC * P], F16,
                          kind="ExternalOutput")

    with tile.TileContext(nc) as tc, ExitStack() as ctx:
        consts = ctx.enter_context(tc.tile_pool(name="consts", bufs=1))
        work = ctx.enter_context(tc.tile_pool(name="work", bufs=1))
        sq_pool = ctx.enter_context(tc.tile_pool(name="sq", bufs=3))
        ps_psi = ctx.enter_context(tc.tile_pool(name="ps_psi", bufs=1,
                                                space="PSUM"))
        ps_a = ctx.enter_context(tc.tile_pool(name="ps_a", bufs=3,
                                              space="PSUM"))
        ps_b = ctx.enter_context(tc.tile_pool(name="ps_b", bufs=2,
                                              space="PSUM"))
        ps_j = ctx.enter_context(tc.tile_pool(name="ps_j", bufs=1,
                                              space="PSUM"))

        # dummy ACT op to trigger the activation-table load during the
        # input-DMA wait instead of at the first real ACT use
        sb_dummy = consts.tile([P, 8], F32, tag="dummy")
        nc.gpsimd.memset(sb_dummy, 0.0)
        nc.scalar.square(sb_dummy, sb_dummy)

        # input DMAs: [Y | V cols 0-383] on sync, rest of V on scalar, then
        # the rotation weights
        sb_vy = consts.tile([P, 9 * P], F16, tag="vy")
        nc.sync.dma_start(out=sb_vy[:, 0:4 * P], in_=d_vy.ap()[:, 0:4 * P])
        nc.scalar.dma_start(out=sb_vy[:, 4 * P:9 * P],
                            in_=d_vy.ap()[:, 4 * P:9 * P])
        sb_wa = consts.tile([P, 2 * BPC * P], F16, tag="wa")
        nc.scalar.dma_start(out=sb_wa, in_=d_wa.ap())
        sb_wf = consts.tile([2 * F, BPC * 2 * F], F16, tag="wf")
        nc.sync.dma_start(out=sb_wf, in_=d_wf.ap())

        # PE warm-up: junk matmuls with no data deps keep the tensor engine
        # busy through the input-DMA wait so its p-state ramps to full clock
        # before the real work arrives
        sb_junk = consts.tile([P, 4 * P], F16, tag="junk")
        nc.gpsimd.memset(sb_junk, 0.0)
        ps_junk = ps_j.tile([P, 4 * P], F32, tag="psj")
        for _ in range(3):
            nc.tensor.matmul(ps_junk, sb_junk[:, 0:P], sb_junk,
                             start=True, stop=True, skip_group_check=True)

        # ---------------- evolution: psi for all 8 t ----------------
        # one PSUM tile, cols (g, j, t, rr); lhsT = V chunk, rhs = Y
        psi = ps_psi.tile([P, 8 * P], F32, tag="psi")
        for g in range(8):
            nc.tensor.matmul(psi[:, g * P:(g + 1) * P],
                             sb_vy[:, (1 + g) * P:(2 + g) * P],
                             sb_vy[:, 0:P], start=True, stop=True,
                             skip_group_check=True)

        # bridge junk matmuls: keep the PE busy through the rr-copy gap so
        # the DVFS ramp isn't reset before stage A
        for _ in range(4):
            nc.tensor.matmul(ps_junk, sb_junk[:, 0:P], sb_junk,
                             start=True, stop=True, skip_group_check=True)

        # rr-copies: PSUM -> fp16 cat tiles, cols (t, rr-pair, q=(g j)).
        # cat1/cat2 are separate tiles each written by ONE engine -- the tile
        # framework serializes same-tile writers across engines otherwise.
        sb_cat1 = work.tile([P, T * 2 * F], F16, tag="cat1")
        sb_cat2 = work.tile([P, T * 2 * F], F16, tag="cat2")
        psi_r = psi.rearrange("p (gj t rr) -> p rr t gj", gj=F, t=T, rr=4)
        c1_r = sb_cat1.rearrange("p (t rr gj) -> p rr t gj", gj=F, t=T, rr=2)
        c2_r = sb_cat2.rearrange("p (t rr gj) -> p rr t gj", gj=F, t=T, rr=2)
        _copy(nc.vector, c1_r[:, 0], psi_r[:, 0])
        _copy(nc.vector, c1_r[:, 1], psi_r[:, 1])
        _copy(nc.scalar, c2_r[:, 0], psi_r[:, 2])
        _copy(nc.scalar, c2_r[:, 1], psi_r[:, 3])

        # ---------------- rotation stage A ----------------
        # psA_t[(r q), (bl p')] = [T_re; T_im]; casts split per observable
        # pair into per-(h, bb) tiles, engine keyed by bb (single writer per
        # tile, both engines run concurrently per t)
        sb_a00 = work.tile([2 * F, 2 * 4 * P], F16, tag="sba00")
        sb_a01 = work.tile([2 * F, 2 * 4 * P], F16, tag="sba01")
        sb_a10 = work.tile([2 * F, 2 * 4 * P], F16, tag="sba10")
        sb_a11 = work.tile([2 * F, 2 * 4 * P], F16, tag="sba11")
        sb_ahb = [[sb_a00, sb_a01], [sb_a10, sb_a11]]
        aeng = [nc.vector, nc.scalar]

        def stage_a(t):
            psA = ps_a.tile([2 * F, BPC * P], F32, tag="psA")
            nc.tensor.matmul(psA, sb_cat1[:, t * 2 * F:(t + 1) * 2 * F],
                             sb_wa[:, 0:BPC * P], start=True, stop=False)
            nc.tensor.matmul(psA, sb_cat2[:, t * 2 * F:(t + 1) * 2 * F],
                             sb_wa[:, BPC * P:2 * BPC * P],
                             start=False, stop=True)
            psA_v = psA.rearrange("p (b x) -> p b x", b=BPC)
            for bb in range(2):
                dst = sb_ahb[t // 4][bb].rearrange(
                    "p (u t x) -> p t u x", u=2, t=4, x=P)[:, t % 4]
                _copy(aeng[bb], dst, psA_v[:, 2 * bb:2 * bb + 2])

        # ---------------- stage B + |.|^2 + out ----------------
        # psB packs two observables on the partition axis (PE quadrant
        # placement); squares on ACT; re^2+im^2 pair-add happens on host
        def stage_b(h, bb):
            psB = ps_b.tile([P, BPC * P], F32, tag="psB")
            for u in range(2):
                nc.tensor.matmul(
                    psB[64 * u:64 * (u + 1), :],
                    sb_wf[:, (2 * bb + u) * 2 * F:(2 * bb + u + 1) * 2 * F],
                    sb_ahb[h][bb][:, u * 4 * P:(u + 1) * 4 * P],
                    start=True, stop=True, skip_group_check=True)
            sq = sq_pool.tile([P, BPC * P], F16, tag="sq")
            nc.scalar.square(sq, psB)
            nc.sync.dma_start(
                out=d_sq.ap()[:, (2 * h + bb) * BPC * P:
                              (2 * h + bb + 1) * BPC * P], in_=sq)

        for t in range(4):
            stage_a(t)
        stage_a(4)
        stage_b(0, 0)
        stage_a(5)
        stage_a(6)
        stage_b(0, 1)
        stage_a(7)
        stage_b(1, 0)
        stage_b(1, 1)

    nc.compile()
    return nc


# ----------------------------------------------------------------------------
# entry point
# ----------------------------------------------------------------------------

_PROGRAM_CACHE = {}

# test-harness knobs (grading path leaves these untouched)
TRACE = False
LAST_RESULT = None


def kernel(initial_state, ts, pauli_obs, indices, params_x, params_zz):
    ts = np.asarray(ts)
    pauli_obs = np.asarray(pauli_obs)
    indices = np.asarray(indices)
    Tn = ts.shape[0]
    shots = indices.shape[2]
    assert Tn == T, f"expected {T} timesteps, got {Tn}"

    shared, per_core = prepare_host_data(
        initial_state, ts, pauli_obs, params_x, params_zz)

    if "prog" not in _PROGRAM_CACHE:
        _PROGRAM_CACHE["prog"] = build_program()
    nc = _PROGRAM_CACHE["prog"]

    in_maps = [{**shared, **pc} for pc in per_core]
    res = run_bass_kernel_spmd(nc, in_maps, core_ids=list(range(NCORES)),
                               trace=TRACE)
    global LAST_RESULT
    LAST_RESULT = res

    out = np.zeros((Tn, B, shots), np.float32)
    idx = indices.astype(np.int64)
    for c in range(NCORES):
        tiles = np.asarray(res.results[c]["sqout"], np.float32)  # (128, 2048)
        # chunk (h, bb) at cols (2h+bb)*512; rows 64u+32r+q'; cols (t%4, p')
        ch = tiles.reshape(P, 2, 2, 4, P).transpose(1, 2, 0, 3, 4)
        ch = ch.reshape(2, 2, 2, 2, F, 4, P)        # [h, bb, u, r, q', t4, p']
        pr = ch.sum(axis=3)                          # re^2 + im^2
        # -> [t, bl, n]: t = 4h + t4, bl = 2bb + u, n = q'<<7 | p'
        pr = pr.transpose(0, 4, 1, 2, 3, 5).reshape(Tn, BPC, DIM)
        for bl in range(BPC):
            b = BPC * c + bl
            out[:, b, :] = np.take_along_axis(pr[:, bl], idx[:, b], axis=1)
    return out


# revision 20
# speedup vs baseline: 1.0815x; 1.0505x over previous
"""Trainium2 Bass kernel for nn_ExactModel_15092515078731.

Reference computes, per timestep t:
    U = expm(-i t H);  psi = U[:, 0]
    rotate psi by 32 per-observable tensor-product single-qubit bases
    probs = |rotated|^2 ; gather at indices

Algorithm here: Krylov (Lanczos) projection.  H is real-symmetric, so
psi_t = expm(-itH) e0 ~= V exp(-itT) e1 with V the (t-independent!)
m=32-vector Lanczos basis of K(H, e0) and T the 32x32 tridiagonal
projection, both built on host (the baseline already ran 80 host
Lanczos iterations just for spectral bounds; this reuses that work).
Per-t coefficient vectors y_t = exp(-itT) e1 are tiny (32 complex).

Device work per core (SPMD over 8 cores, sharded by OBSERVABLE --
each core owns 4 of the 32 observables for all 8 timesteps):
  1. evolution: psi_{t,r}[p, q] = sum_k V[(q<<7)|p, k] y^r_t[k] for all
     8 t and r in {re, im, -im, re} -- 8 fp16 matmuls of 128 cols via a
     block-diagonal y trick (4 q-values x 32 k on the contraction
     partitions).
  2. rotation stage A (SWAPPED operands: state stationary, weights
     moving -- no transposes needed): psA_t[(r q), (b p')] accumulates
     cat1_t^T Wre + cat2_t^T Wim = [T_re; T_im] stacked on partitions,
     2 matmuls of 512 cols per t.
  3. rotation stage B: 64x64 complex-structured Wfree block per b,
     rhs = fp16 cast of psA in [64, (b t p')] layout; out [64, (t p')]
     = [F_re; F_im].
  4. |.|^2: square on ACT/DVE/Pool; partition pair-add moved to the PE
     as accumulating 0/1 "pair-sum" matmuls that also pack the 4
     observables onto 128 partitions for a full-width output DMA.
Host does only small parameter prep (Lanczos on one 4096-vector,
rotation kron products) and the final index gather.
"""
import sys

if "/opt/trn_rl_repo" not in sys.path:
    sys.path.insert(0, "/opt/trn_rl_repo")

from contextlib import ExitStack

import numpy as np

import concourse.bacc as bacc
import concourse.bass as bass  # noqa: F401
import concourse.mybir as mybir
import concourse.tile as tile
from concourse.bass_utils import run_bass_kernel_spmd

N = 12
DIM = 4096
P = 128    # partition: bits 0-6
F = 32     # free: bits 7-11
NCORES = 8
B = 32     # observables
BPC = 4    # observables per core
T = 8      # timesteps
M = 32     # Krylov dimension

_s = 1.0 / np.sqrt(2.0)
U_BASIS = np.stack([
    np.array([[1, 1], [1, -1]]) * _s,
    np.array([[1, -1j], [1, 1j]]) * _s,
    np.eye(2),
]).astype(np.complex128)

F32 = mybir.dt.float32
F16 = mybir.dt.float16
MULT = mybir.AluOpType.mult
ADD = mybir.AluOpType.add


# ----------------------------------------------------------------------------
# host math
# ----------------------------------------------------------------------------

def _build_zz_diag(params_zz):
    basis = np.arange(DIM)
    bits = (basis[:, None] >> np.arange(N)[None, :]) & 1
    signs = (1 - 2 * bits).astype(np.float64)
    return (signs[:, :-1] * signs[:, 1:]) @ params_zz


def _h_matvec(v, params_x, zz_diag):
    out = zz_diag * v
    idx = np.arange(DIM)
    for i in range(N):
        out = out + params_x[i] * v[idx ^ (1 << i)]
    return out


def _lanczos(params_x, zz_diag, m=M):
    """m-step Lanczos of H from e0 with full reorthogonalization.
    Returns V (DIM, m).  On breakdown the remaining columns stay zero
    (the Krylov space is then invariant and the projection exact)."""
    V = np.zeros((DIM, m))
    V[0, 0] = 1.0
    for j in range(m - 1):
        w = _h_matvec(V[:, j], params_x, zz_diag)
        for _ in range(2):
            w = w - V[:, :j + 1] @ (V[:, :j + 1].T @ w)
        beta = np.linalg.norm(w)
        if beta < 1e-10:
            break
        V[:, j + 1] = w / beta
    return V


def _build_rot_mats(pauli_obs):
    """Wpart (B,128,128), Wfree (B,32,32); qubit acting on bit k is
    U_BASIS[pauli_obs[b, 11-k]] (reference reshape is bit-11-major)."""
    Wpart = np.zeros((B, P, P), np.complex128)
    Wfree = np.zeros((B, F, F), np.complex128)
    for b in range(B):
        Ub = [U_BASIS[pauli_obs[b, 11 - k]] for k in range(N)]
        wp = np.array([[1.0]])
        for k in range(6, -1, -1):
            wp = np.kron(wp, Ub[k])
        wf = np.array([[1.0]])
        for k in range(11, 6, -1):
            wf = np.kron(wf, Ub[k])
        Wpart[b] = wp
        Wfree[b] = wf
    return Wpart, Wfree


def prepare_host_data(initial_state, ts, pauli_obs, params_x, params_zz):
    """Returns (shared dict, per-core list of dicts)."""
    n0 = int(initial_state)
    assert n0 == 0
    ts = np.asarray(ts, np.float64)
    pauli_obs = np.asarray(pauli_obs, np.int64)
    params_x = np.asarray(params_x, np.float64)
    params_zz = np.asarray(params_zz, np.float64)

    zz_diag = _build_zz_diag(params_zz)
    V = _lanczos(params_x, zz_diag)                       # (DIM, M)
    HV = np.stack([_h_matvec(V[:, k], params_x, zz_diag)
                   for k in range(M)], axis=1)
    Tm = V.T @ HV                                          # (M, M)
    wT, QT = np.linalg.eigh(Tm)
    # y_t = exp(-i t T) e1
    ys = [QT @ (np.exp(-1j * t * wT) * QT[0, :]) for t in ts]

    # V in evolution lhsT layout: V16[(j,k), g*128+p] = V[((4g+j)<<7)|p, k]
    Vr = V.reshape(F, P, M)                                # [q, p, k]
    V16 = np.zeros((P, 8 * P), np.float16)
    for g in range(8):
        for j in range(4):
            # rows j*32+k, cols g*128+p
            V16[j * M:(j + 1) * M, g * P:(g + 1) * P] = \
                Vr[4 * g + j].T.astype(np.float16)
    # Y block-diag: Y[(j,k), j'*32 + t*4 + rr] = (j==j') * y^rr_t[k]
    Y16 = np.zeros((P, P), np.float16)
    for t in range(T):
        yre = ys[t].real
        yim = ys[t].imag
        for j in range(4):
            rows = np.s_[j * M:(j + 1) * M]
            Y16[rows, j * M + t * 4 + 0] = yre.astype(np.float16)
            Y16[rows, j * M + t * 4 + 1] = yim.astype(np.float16)
            Y16[rows, j * M + t * 4 + 2] = (-yim).astype(np.float16)
            Y16[rows, j * M + t * 4 + 3] = yre.astype(np.float16)
    VY = np.concatenate([Y16, V16], axis=1)                # (128, 1152)

    Wpart, Wfree = _build_rot_mats(pauli_obs)
    per_core = []
    for c in range(NCORES):
        bs = [BPC * c + i for i in range(BPC)]
        # stage A moving weights: WA[p, w*512 + bl*128 + p'] = re/im W[p',p]
        WA = np.zeros((P, 2 * BPC * P), np.float16)
        for bl, b in enumerate(bs):
            WA[:, 0 * BPC * P + bl * P:(0 * BPC + bl + 1) * P] = \
                Wpart[b].T.real.astype(np.float16)
            WA[:, 1 * BPC * P + bl * P:1 * BPC * P + (bl + 1) * P] = \
                Wpart[b].T.imag.astype(np.float16)
        # stage B stationary: complex-structured 64x64 block per bl
        WF = np.zeros((2 * F, BPC * 2 * F), np.float16)
        for bl, b in enumerate(bs):
            fre = Wfree[b].real.T.astype(np.float16)   # [q, q'] = Wf[q',q]
            fim = Wfree[b].imag.T.astype(np.float16)
            blk = np.s_[bl * 2 * F: bl * 2 * F + 2 * F]
            WF[0:F, blk][:, 0:F] = fre
            WF[0:F, blk][:, F:2 * F] = fim
            WF[F:2 * F, blk][:, 0:F] = -fim
            WF[F:2 * F, blk][:, F:2 * F] = fre
        per_core.append(dict(wa=WA, wf=WF))
    shared = dict(vy=VY)
    return shared, per_core


# ----------------------------------------------------------------------------
# device program
# ----------------------------------------------------------------------------

def _copy(eng, out, in_):
    if hasattr(eng, "tensor_copy"):
        eng.tensor_copy(out, in_)
    else:
        eng.copy(out, in_)


def build_program():
    nc = bacc.Bacc("TRN2", target_bir_lowering=False, debug=False,
                   num_devices=NCORES)

    d_vy = nc.dram_tensor("vy", [P, 9 * P], F16, kind="ExternalInput")
    d_wa = nc.dram_tensor("wa", [P, 2 * BPC * P], F16, kind="ExternalInput")
    d_wf = nc.dram_tensor("wf", [2 * F, BPC * 2 * F], F16,
                          kind="ExternalInput")
    d_sq = nc.dram_tensor("sqout", [P, 4 * BPC * P], F16,
                          kind="ExternalOutput")

    with tile.TileContext(nc) as tc, ExitStack() as ctx:
        consts = ctx.enter_context(tc.tile_pool(name="consts", bufs=1))
        work = ctx.enter_context(tc.tile_pool(name="work", bufs=1))
        sq_pool = ctx.enter_context(tc.tile_pool(name="sq", bufs=3))
        ps_psi = ctx.enter_context(tc.tile_pool(name="ps_psi", bufs=1,
                                                space="PSUM"))
        ps_a = ctx.enter_context(tc.tile_pool(name="ps_a", bufs=4,
                                              space="PSUM"))
        ps_b = ctx.enter_context(tc.tile_pool(name="ps_b", bufs=2,
                                              space="PSUM"))

        # dummy ACT op to trigger the activation-table load during the
        # input-DMA wait instead of at the first real ACT use
        sb_dummy = consts.tile([P, 8], F32, tag="dummy")
        nc.gpsimd.memset(sb_dummy, 0.0)
        nc.scalar.square(sb_dummy, sb_dummy)

        # input DMAs: [Y | V cols 0-383] on sync, rest of V on scalar, then
        # the rotation weights
        sb_vy = consts.tile([P, 9 * P], F16, tag="vy")
        nc.sync.dma_start(out=sb_vy[:, 0:5 * P], in_=d_vy.ap()[:, 0:5 * P])
        nc.scalar.dma_start(out=sb_vy[:, 5 * P:9 * P],
                            in_=d_vy.ap()[:, 5 * P:9 * P])
        sb_wa = consts.tile([P, 2 * BPC * P], F16, tag="wa")
        nc.scalar.dma_start(out=sb_wa, in_=d_wa.ap())
        sb_wf = consts.tile([2 * F, BPC * 2 * F], F16, tag="wf")
        nc.sync.dma_start(out=sb_wf, in_=d_wf.ap())

        # PE warm-up: junk matmuls with no data deps keep the tensor engine
        # busy through the input-DMA wait so its p-state ramps to full clock
        # before the real work arrives
        sb_junk = consts.tile([P, 4 * P], F16, tag="junk")
        nc.gpsimd.memset(sb_junk, 0.0)
        ps_junk = ps_b.tile([P, 4 * P], F32, tag="psB")
        for _ in range(3):
            nc.tensor.matmul(ps_junk, sb_junk[:, 0:P], sb_junk,
                             start=True, stop=True, skip_group_check=True)

        # ---------------- evolution: psi for all 8 t ----------------
        # one PSUM tile, cols (g, j, t, rr); lhsT = V chunk, rhs = Y
        psi = ps_psi.tile([P, 8 * P], F32, tag="psi")
        for g in range(8):
            nc.tensor.matmul(psi[:, g * P:(g + 1) * P],
                             sb_vy[:, (1 + g) * P:(2 + g) * P],
                             sb_vy[:, 0:P], start=True, stop=True,
                             skip_group_check=True)

        # bridge junk matmuls: keep the PE busy through the rr-copy gap so
        # the DVFS ramp isn't reset before stage A
        for _ in range(4):
            nc.tensor.matmul(ps_junk, sb_junk[:, 0:P], sb_junk,
                             start=True, stop=True, skip_group_check=True)

        # rr-copies: PSUM -> fp16 cat tiles, cols (t, rr-pair, q=(g j)).
        # cat1/cat2 are separate tiles each written by ONE engine -- the tile
        # framework serializes same-tile writers across engines otherwise.
        sb_cat1 = work.tile([P, T * 2 * F], F16, tag="cat1")
        sb_cat2 = work.tile([P, T * 2 * F], F16, tag="cat2")
        psi_r = psi.rearrange("p (gj t rr) -> p rr t gj", gj=F, t=T, rr=4)
        c1_r = sb_cat1.rearrange("p (t rr gj) -> p rr t gj", gj=F, t=T, rr=2)
        c2_r = sb_cat2.rearrange("p (t rr gj) -> p rr t gj", gj=F, t=T, rr=2)
        _copy(nc.vector, c1_r[:, 0], psi_r[:, 0])
        _copy(nc.vector, c1_r[:, 1], psi_r[:, 1])
        _copy(nc.vector, c2_r[:, 0], psi_r[:, 2])
        _copy(nc.vector, c2_r[:, 1], psi_r[:, 3])

        # ---------------- rotation stage A ----------------
        # psA_t[(r q), (bl p')] = [T_re; T_im]; casts split per observable
        # pair into per-(h, bb) tiles, engine keyed by bb (single writer per
        # tile, both engines run concurrently per t)
        sb_a00 = work.tile([2 * F, 2 * 4 * P], F16, tag="sba00")
        sb_a01 = work.tile([2 * F, 2 * 4 * P], F16, tag="sba01")
        sb_a10 = work.tile([2 * F, 2 * 4 * P], F16, tag="sba10")
        sb_a11 = work.tile([2 * F, 2 * 4 * P], F16, tag="sba11")
        sb_ahb = [[sb_a00, sb_a01], [sb_a10, sb_a11]]
        aeng = [nc.vector, nc.scalar]

        def stage_a(t):
            psA = ps_a.tile([2 * F, BPC * P], F32, tag="psA")
            nc.tensor.matmul(psA, sb_cat1[:, t * 2 * F:(t + 1) * 2 * F],
                             sb_wa[:, 0:BPC * P], start=True, stop=False)
            nc.tensor.matmul(psA, sb_cat2[:, t * 2 * F:(t + 1) * 2 * F],
                             sb_wa[:, BPC * P:2 * BPC * P],
                             start=False, stop=True)
            psA_v = psA.rearrange("p (b x) -> p b x", b=BPC)
            for bb in range(2):
                dst = sb_ahb[t // 4][bb].rearrange(
                    "p (u t x) -> p t u x", u=2, t=4, x=P)[:, t % 4]
                _copy(aeng[bb], dst, psA_v[:, 2 * bb:2 * bb + 2])

        # ---------------- stage B + |.|^2 + out ----------------
        # psB packs two observables on the partition axis (PE quadrant
        # placement); squares on ACT; re^2+im^2 pair-add happens on host
        def stage_b(h, bb):
            psB = ps_b.tile([P, BPC * P], F32, tag="psB")
            for u in range(2):
                nc.tensor.matmul(
                    psB[64 * u:64 * (u + 1), :],
                    sb_wf[:, (2 * bb + u) * 2 * F:(2 * bb + u + 1) * 2 * F],
                    sb_ahb[h][bb][:, u * 4 * P:(u + 1) * 4 * P],
                    start=True, stop=True, skip_group_check=True)
            sq = sq_pool.tile([P, BPC * P], F16, tag="sq")
            nc.scalar.square(sq, psB)
            nc.sync.dma_start(
                out=d_sq.ap()[:, (2 * h + bb) * BPC * P:
                              (2 * h + bb + 1) * BPC * P], in_=sq)

        for t in range(4):
            stage_a(t)
        stage_a(4)
        stage_b(0, 0)
        stage_a(5)
        stage_a(6)
        stage_b(0, 1)
        stage_a(7)
        stage_b(1, 0)
        stage_b(1, 1)

    nc.compile()
    return nc


# ----------------------------------------------------------------------------
# entry point
# ----------------------------------------------------------------------------

_PROGRAM_CACHE = {}

# test-harness knobs (grading path leaves these untouched)
TRACE = False
LAST_RESULT = None


def kernel(initial_state, ts, pauli_obs, indices, params_x, params_zz):
    ts = np.asarray(ts)
    pauli_obs = np.asarray(pauli_obs)
    indices = np.asarray(indices)
    Tn = ts.shape[0]
    shots = indices.shape[2]
    assert Tn == T, f"expected {T} timesteps, got {Tn}"

    shared, per_core = prepare_host_data(
        initial_state, ts, pauli_obs, params_x, params_zz)

    if "prog" not in _PROGRAM_CACHE:
        _PROGRAM_CACHE["prog"] = build_program()
    nc = _PROGRAM_CACHE["prog"]

    in_maps = [{**shared, **pc} for pc in per_core]
    res = run_bass_kernel_spmd(nc, in_maps, core_ids=list(range(NCORES)),
                               trace=TRACE)
    global LAST_RESULT
    LAST_RESULT = res

    out = np.zeros((Tn, B, shots), np.float32)
    idx = indices.astype(np.int64)
    for c in range(NCORES):
        tiles = np.asarray(res.results[c]["sqout"], np.float32)  # (128, 2048)
        # chunk (h, bb) at cols (2h+bb)*512; rows 64u+32r+q'; cols (t%4, p')
        ch = tiles.reshape(P, 2, 2, 4, P).transpose(1, 2, 0, 3, 4)
        ch = ch.reshape(2, 2, 2, 2, F, 4, P)        # [h, bb, u, r, q', t4, p']
        pr = ch.sum(axis=3)                          # re^2 + im^2
        # -> [t, bl, n]: t = 4h + t4, bl = 2bb + u, n = q'<<7 | p'
        pr = pr.transpose(0, 4, 1, 2, 3, 5).reshape(Tn, BPC, DIM)
        for bl in range(BPC):
            b = BPC * c + bl
            out[:, b, :] = np.take_along_axis(pr[:, bl], idx[:, b], axis=1)
    return out
